# revision 1
# baseline (speedup 1.0000x reference)
"""RGCN (2-layer, mean aggr) + global mean pool on 8 TRN2 NeuronCores.

Sharding: nodes split contiguously across 8 cores (batch-sorted, so the graph
pool shards too); each core owns its incoming edges, bucketed into
(128-node range, relation) windows padded to a fixed tile count. Segment sums
run on the tensor engine as S_tile.T @ gathered_messages with PSUM
accumulation per window (S = host-built 0/1 selector tiles). Edge messages are
fetched with per-tile indirect DMA gathers (128 rows/instruction).
Phase A computes the small layer-1 aggregate mean1 sharded; the host
all-gathers it (~320KB/core) between the two NEFF runs. Phase B recomputes
dense h for all nodes (cheap matmuls on replicated mean1), stores an h table
in DRAM, gathers layer-2 messages, segment-sums, applies the relation einsum
+ root + bias, and pools per-graph partials; the host sums 8 partials.
"""

import numpy as np

import concourse.bacc as bacc
import concourse.bass as bass
import concourse.mybir as mybir
import concourse.tile as tile
from concourse.bass_utils import run_bass_kernel_spmd

N = 10000
E = 160000
R = 4
IN = 15
HID = 512
G = 64
C = 8
NPC = N // C            # 1250 nodes per core
RANGES = 10             # 128-node ranges per core
NPAD = RANGES * 128     # 1280
NTOT = 10112            # 79*128 covers all nodes for dense h
NCH = NTOT // 128
K1 = IN + R * IN + 1    # 76 contract rows for dense h (x, 4 rels, bias)
F32 = mybir.dt.float32
BF16 = mybir.dt.bfloat16
I32 = mybir.dt.int32
Relu = mybir.ActivationFunctionType.Relu

_CACHE = {}


# ---------------------------------------------------------------- host prep
def _prep_structure(edge_index, edge_type, batch):
    src = np.asarray(edge_index[0], dtype=np.int64)
    tgt = np.asarray(edge_index[1], dtype=np.int64)
    rel = np.asarray(edge_type, dtype=np.int64)
    batch = np.asarray(batch, dtype=np.int64)

    core = tgt // NPC
    loc = tgt - core * NPC
    rg = loc // 128
    col = loc % 128
    win = (core * RANGES + rg) * R + rel            # 0..C*40-1
    nwin_core = RANGES * R

    wcount = np.bincount(win, minlength=C * nwin_core)
    t_w = max(5, int(-(-wcount.max() // 128)))      # tiles per window
    slots_w = t_w * 128
    slots_core = nwin_core * slots_w
    tiles_core = nwin_core * t_w

    order = np.lexsort((src, win))
    swin = win[order]
    ssrc = src[order]
    scol = col[order]
    wstart = np.zeros(C * nwin_core + 1, np.int64)
    np.cumsum(wcount, out=wstart[1:])
    pos = np.arange(E) - wstart[swin]
    slot_global = swin * slots_w + pos

    idx_flat = np.zeros(C * slots_core, np.int32)
    colarr = np.zeros(C * slots_core, np.int32)
    valid = np.zeros(C * slots_core, bool)
    idx_flat[slot_global] = ssrc.astype(np.int32)
    colarr[slot_global] = scol
    valid[slot_global] = True

    idx_flat = idx_flat.reshape(C, slots_core)
    colarr = colarr.reshape(C, slots_core)
    valid = valid.reshape(C, slots_core)

    # S tiles [tiles_core, 128, 128] f32, then device layout [RANGES,128,npr*128]
    S = np.zeros((C, tiles_core, 128, 128), np.float32)
    tidx = np.arange(slots_core) // 128
    pidx = np.arange(slots_core) % 128
    for c in range(C):
        v = valid[c]
        S[c, tidx[v], pidx[v], colarr[c][v]] = 1.0

    # per-tile offset columns [128, tiles_core] int32 (slot p of tile t)
    idx_cols = np.ascontiguousarray(
        idx_flat.reshape(C, tiles_core, 128).transpose(0, 2, 1))

    cnt = np.bincount(tgt * R + rel, minlength=N * R).reshape(N, R)
    cntinv = np.zeros((C, 128, nwin_core), np.float32)
    for c in range(C):
        for rgi in range(RANGES):
            n0 = c * NPC + rgi * 128
            nn = np.arange(n0, n0 + 128)
            ok = nn < (c + 1) * NPC
            for r in range(R):
                cv = np.where(ok, np.maximum(cnt[np.minimum(nn, N - 1), r], 1), 1)
                cntinv[c, :, rgi * R + r] = 1.0 / cv

    gcnt = np.bincount(batch, minlength=G)
    ginv = (1.0 / np.maximum(gcnt, 1)).astype(np.float32).reshape(G, 1)
    poolS = np.zeros((C, 128, RANGES, G), np.float32)
    for c in range(C):
        for rgi in range(RANGES):
            n0 = c * NPC + rgi * 128
            nn = np.arange(n0, min(n0 + 128, (c + 1) * NPC))
            if len(nn):
                poolS[c, np.arange(len(nn)), rgi, batch[nn]] = 1.0
    poolS = poolS.reshape(C, 128, RANGES * G)

    return dict(t_w=t_w, tiles_core=tiles_core, slots_core=slots_core,
                idx_cols=idx_cols, S=S, cntinv=cntinv, poolS=poolS, ginv=ginv)


def _s_dev(s_core):
    tiles_core = s_core.shape[0]
    npr = tiles_core // RANGES
    return np.ascontiguousarray(
        s_core.reshape(RANGES, npr, 128, 128).transpose(0, 2, 1, 3)
        .reshape(RANGES, 128, npr * 128))


# ---------------------------------------------------------------- phase A
def _build_phase_a(t_w):
    tiles_core = RANGES * R * t_w
    npr = R * t_w
    nc = bacc.Bacc("TRN2", target_bir_lowering=True)
    xg_d = nc.dram_tensor("xg", [RANGES, 128, (R * t_w) * 16], BF16,
                          kind="ExternalInput")
    s_d = nc.dram_tensor("stab", [RANGES, 128, npr * 128], BF16,
                         kind="ExternalInput")
    ci_d = nc.dram_tensor("cntinv", [128, RANGES * R], F32, kind="ExternalInput")
    out_d = nc.dram_tensor("mean1", [RANGES * R * 128, 16], F32,
                           kind="ExternalOutput")

    with tile.TileContext(nc) as tc:
        with (
            tc.tile_pool(name="singles", bufs=1) as singles,
            tc.tile_pool(name="gbuf", bufs=8) as gpool,
            tc.tile_pool(name="sbufS", bufs=2) as spool,
            tc.tile_pool(name="m1", bufs=4) as mpool,
            tc.tile_pool(name="ps", bufs=4, space="PSUM") as pspool,
        ):
            ci_sb = singles.tile([128, RANGES * R], F32)
            nc.sync.dma_start(out=ci_sb[:], in_=ci_d[:])
            for rgi in range(RANGES):
                st = spool.tile([128, npr, 128], BF16, tag="s")
                nc.sync.dma_start(out=st[:],
                                  in_=s_d[rgi].rearrange("p (t c) -> p t c", c=128))
                gt = gpool.tile([128, npr, 16], BF16, tag="g")
                nc.sync.dma_start(out=gt[:],
                                  in_=xg_d[rgi].rearrange("p (t c) -> p t c", c=16))
                for r in range(R):
                    ps = pspool.tile([128, 16], F32)
                    for t in range(t_w):
                        k = r * t_w + t
                        nc.tensor.matmul(ps[:], lhsT=st[:, k, :], rhs=gt[:, k, :],
                                         start=(t == 0), stop=(t == t_w - 1))
                    w = rgi * R + r
                    m1 = mpool.tile([128, 16], F32)
                    nc.vector.tensor_scalar_mul(m1[:], ps[:], ci_sb[:, w:w + 1])
                    nc.sync.dma_start(out=out_d[w * 128:(w + 1) * 128, :], in_=m1[:])
    nc.compile()
    return nc


# ---------------------------------------------------------------- phase B
def _build_phase_b(t_w):
    tiles_core = RANGES * R * t_w
    npr = R * t_w
    nc = bacc.Bacc("TRN2", target_bir_lowering=True)
    m1xT_d = nc.dram_tensor("m1xT", [K1, NTOT], F32, kind="ExternalInput")
    m1own_d = nc.dram_tensor("m1own", [K1, NPAD], F32, kind="ExternalInput")
    w1_d = nc.dram_tensor("w1all", [K1, HID], F32, kind="ExternalInput")
    w2f_d = nc.dram_tensor("w2flat", [128, 16 * HID], BF16, kind="ExternalInput")
    w2r_d = nc.dram_tensor("w2root", [128, 4 * HID], F32, kind="ExternalInput")
    b2_d = nc.dram_tensor("b2row", [1, HID], F32, kind="ExternalInput")
    idx_d = nc.dram_tensor("idx", [128, tiles_core], I32, kind="ExternalInput")
    s_d = nc.dram_tensor("stab", [RANGES, 128, npr * 128], BF16,
                         kind="ExternalInput")
    ci_d = nc.dram_tensor("cntinv", [128, RANGES * R], F32, kind="ExternalInput")
    pS_d = nc.dram_tensor("poolS", [128, RANGES * G], F32, kind="ExternalInput")
    gi_d = nc.dram_tensor("ginv", [G, 1], F32, kind="ExternalInput")
    id_d = nc.dram_tensor("ident", [128, 128], BF16, kind="ExternalInput")
    out_d = nc.dram_tensor("pooled", [G, HID], F32, kind="ExternalOutput")

    with tile.TileContext(nc) as tc:
        with (
            tc.tile_pool(name="singles", bufs=1) as singles,
            tc.tile_pool(name="dram", bufs=1, space="DRAM") as dpool,
            tc.tile_pool(name="hb", bufs=3) as hpool,
            tc.tile_pool(name="gbuf", bufs=8) as gpool,
            tc.tile_pool(name="sbufS", bufs=2) as spool,
            tc.tile_pool(name="mb", bufs=3) as mbpool,
            tc.tile_pool(name="mt", bufs=2) as mtpool,
            tc.tile_pool(name="ob", bufs=2) as opool,
            tc.tile_pool(name="ph", bufs=2, space="PSUM") as php,
            tc.tile_pool(name="pm", bufs=2, space="PSUM") as pmp,
            tc.tile_pool(name="po", bufs=1, space="PSUM") as pop,
            tc.tile_pool(name="pp", bufs=1, space="PSUM") as ppp,
            tc.tile_pool(name="pt", bufs=2, space="PSUM") as ptp,
        ):
            m1xT = singles.tile([K1, NTOT], F32)
            nc.sync.dma_start(out=m1xT[:], in_=m1xT_d[:])
            m1own = singles.tile([K1, NPAD], F32)
            nc.sync.dma_start(out=m1own[:], in_=m1own_d[:])
            w1 = singles.tile([K1, HID], F32)
            nc.sync.dma_start(out=w1[:], in_=w1_d[:])
            w2f = singles.tile([128, 16 * HID], BF16)
            nc.sync.dma_start(out=w2f[:], in_=w2f_d[:])
            w2r = singles.tile([128, 4 * HID], F32)
            nc.sync.dma_start(out=w2r[:], in_=w2r_d[:])
            b2 = singles.tile([1, HID], F32)
            nc.sync.dma_start(out=b2[:], in_=b2_d[:])
            ones = singles.tile([1, 128], F32)
            nc.vector.memset(ones[:], 1.0)
            idx_sb = singles.tile([128, tiles_core], I32)
            nc.sync.dma_start(out=idx_sb[:], in_=idx_d[:])
            ci_sb = singles.tile([128, RANGES * R], F32)
            nc.sync.dma_start(out=ci_sb[:], in_=ci_d[:])
            pS = singles.tile([128, RANGES * G], F32)
            nc.sync.dma_start(out=pS[:], in_=pS_d[:])
            gi = singles.tile([G, 1], F32)
            nc.sync.dma_start(out=gi[:], in_=gi_d[:])
            ident = singles.tile([128, 128], BF16)
            nc.sync.dma_start(out=ident[:], in_=id_d[:])
            hown = singles.tile([128, 4 * NPAD], F32)

            h_tab = dpool.tile([NTOT, HID], BF16)

            for ch in range(NCH):
                ph = php.tile([128, HID], F32, tag="ph")
                nc.tensor.matmul(ph[:], lhsT=m1xT[:, ch * 128:(ch + 1) * 128],
                                 rhs=w1[:], start=True, stop=True)
                hb = hpool.tile([128, HID], BF16)
                nc.scalar.activation(hb[:], ph[:], Relu)
                nc.sync.dma_start(out=h_tab[ch * 128:(ch + 1) * 128, :], in_=hb[:])
            for hc in range(4):
                for o, wdt in ((0, 512), (512, 512), (1024, 256)):
                    ph = php.tile([128, HID], F32, tag="ph")
                    nc.tensor.matmul(ph[:, :wdt],
                                     lhsT=w1[:, hc * 128:(hc + 1) * 128],
                                     rhs=m1own[:, o:o + wdt], start=True, stop=True)
                    nc.scalar.activation(
                        hown[:, hc * NPAD + o:hc * NPAD + o + wdt], ph[:, :wdt],
                        Relu)

            pool_ps = ppp.tile([G, HID], F32)
            for rgi in range(RANGES):
                st = spool.tile([128, npr, 128], BF16, tag="s")
                nc.sync.dma_start(out=st[:],
                                  in_=s_d[rgi].rearrange("p (t c) -> p t c", c=128))
                mt = mtpool.tile([128, 16 * 128], BF16, tag="mt")
                for r in range(R):
                    pm = pmp.tile([128, HID], F32, tag="pm")
                    for t in range(t_w):
                        k = r * t_w + t
                        gt = gpool.tile([128, HID], BF16, tag="g")
                        nc.gpsimd.indirect_dma_start(
                            out=gt[:], out_offset=None, in_=h_tab[:, :],
                            in_offset=bass.IndirectOffsetOnAxis(
                                ap=idx_sb[:, rgi * npr + k:rgi * npr + k + 1],
                                axis=0))
                        nc.tensor.matmul(pm[:], lhsT=st[:, k, :], rhs=gt[:],
                                         start=(t == 0), stop=(t == t_w - 1))
                    w = rgi * R + r
                    mb = mbpool.tile([128, HID], BF16, tag="mb")
                    nc.vector.tensor_scalar_mul(mb[:], pm[:], ci_sb[:, w:w + 1])
                    for hc in range(4):
                        pt = ptp.tile([128, 128], BF16, tag="pt")
                        nc.tensor.transpose(pt[:], mb[:, hc * 128:(hc + 1) * 128],
                                            ident[:])
                        nc.vector.tensor_copy(
                            out=mt[:, (r * 4 + hc) * 128:(r * 4 + hc + 1) * 128],
                            in_=pt[:])
                po = pop.tile([128, HID], F32, tag="po")
                for k in range(16):
                    nc.tensor.matmul(po[:], lhsT=mt[:, k * 128:(k + 1) * 128],
                                     rhs=w2f[:, k * HID:(k + 1) * HID],
                                     start=(k == 0), stop=False)
                for hc in range(4):
                    nc.tensor.matmul(
                        po[:],
                        lhsT=hown[:, hc * NPAD + rgi * 128:
                                  hc * NPAD + (rgi + 1) * 128],
                        rhs=w2r[:, hc * HID:(hc + 1) * HID],
                        start=False, stop=False)
                nc.tensor.matmul(po[:], lhsT=ones[:, :], rhs=b2[:],
                                 start=False, stop=True)
                o2 = opool.tile([128, HID], F32, tag="o2")
                nc.scalar.activation(o2[:], po[:], Relu)
                nc.tensor.matmul(pool_ps[:], lhsT=pS[:, rgi * G:(rgi + 1) * G],
                                 rhs=o2[:], start=(rgi == 0),
                                 stop=(rgi == RANGES - 1))
            pooled = opool.tile([G, HID], F32, tag="pooled")
            nc.vector.tensor_scalar_mul(pooled[:], pool_ps[:], gi[:, 0:1])
            nc.sync.dma_start(out=out_d[:], in_=pooled[:])
    nc.compile()
    return nc


# ---------------------------------------------------------------- driver
def kernel(x, edge_index, edge_type, batch, W1_rel, W1_root, b1,
           W2_rel, W2_root, b2, _collect_times=None):
    x = np.asarray(x, np.float32)
    W1_rel = np.asarray(W1_rel, np.float32)
    W1_root = np.asarray(W1_root, np.float32)
    b1 = np.asarray(b1, np.float32)
    W2_rel = np.asarray(W2_rel, np.float32)
    W2_root = np.asarray(W2_root, np.float32)
    b2 = np.asarray(b2, np.float32)

    st = _prep_structure(edge_index, edge_type, batch)
    t_w = st["t_w"]

    if ("A", t_w) not in _CACHE:
        _CACHE[("A", t_w)] = _build_phase_a(t_w)
    if ("B", t_w) not in _CACHE:
        _CACHE[("B", t_w)] = _build_phase_b(t_w)
    nca, ncb = _CACHE[("A", t_w)], _CACHE[("B", t_w)]

    import ml_dtypes
    xpad = np.zeros((N, 16), np.float32)
    xpad[:, :IN] = x
    t_c = st["tiles_core"]
    npr = t_c // RANGES

    def _xg(c):
        idx = st["idx_cols"][c]                      # [128, tiles]
        g = xpad[idx.T.reshape(-1)].reshape(t_c, 128, 16)
        return np.ascontiguousarray(
            g.reshape(RANGES, npr, 128, 16).transpose(0, 2, 1, 3)
            .reshape(RANGES, 128, npr * 16)).astype(ml_dtypes.bfloat16)

    in_maps_a = [{
        "xg": _xg(c),
        "stab": _s_dev(st["S"][c]).astype(ml_dtypes.bfloat16),
        "cntinv": np.ascontiguousarray(st["cntinv"][c]),
    } for c in range(C)]
    import time as _time
    _t0 = _time.time()
    ra = run_bass_kernel_spmd(nca, in_maps_a, core_ids=list(range(C)))
    if _collect_times is not None:
        _collect_times.append(int((_time.time() - _t0) * 1e9))

    mean1 = np.zeros((N, R, IN), np.float32)
    for c in range(C):
        mo = np.asarray(ra.results[c]["mean1"]).reshape(RANGES, R, 128, 16)
        for rgi in range(RANGES):
            n0 = c * NPC + rgi * 128
            n1 = min(n0 + 128, (c + 1) * NPC)
            if n1 > n0:
                mean1[n0:n1] = mo[rgi, :, :n1 - n0, :IN].transpose(1, 0, 2)

    m1xT = np.zeros((K1, NTOT), np.float32)
    m1xT[:IN, :N] = x.T
    for r in range(R):
        m1xT[IN + r * IN:IN + (r + 1) * IN, :N] = mean1[:, r, :].T
    m1xT[K1 - 1, :N] = 1.0
    w1all = np.concatenate(
        [W1_root, W1_rel.reshape(R * IN, HID), b1.reshape(1, HID)], 0)
    w2flat = np.ascontiguousarray(
        W2_rel.reshape(16, 128, HID).transpose(1, 0, 2)
        .reshape(128, 16 * HID)).astype(ml_dtypes.bfloat16)
    w2root = np.ascontiguousarray(
        W2_root.reshape(4, 128, HID).transpose(1, 0, 2).reshape(128, 4 * HID))

    in_maps_b = []
    for c in range(C):
        ob = c * NPC
        m1own = np.zeros((K1, NPAD), np.float32)
        m1own[:, :min(NPAD, NTOT - ob)] = m1xT[:, ob:ob + NPAD]
        in_maps_b.append({
            "m1xT": m1xT, "m1own": m1own, "w1all": w1all,
            "w2flat": w2flat, "w2root": w2root,
            "b2row": b2.reshape(1, HID),
            "idx": st["idx_cols"][c],
            "stab": _s_dev(st["S"][c]).astype(ml_dtypes.bfloat16),
            "cntinv": np.ascontiguousarray(st["cntinv"][c]),
            "poolS": np.ascontiguousarray(st["poolS"][c]),
            "ginv": st["ginv"],
            "ident": np.eye(128, dtype=np.float32).astype(ml_dtypes.bfloat16),
        })
    _t0 = _time.time()
    rb = run_bass_kernel_spmd(ncb, in_maps_b, core_ids=list(range(C)))
    if _collect_times is not None:
        _collect_times.append(int((_time.time() - _t0) * 1e9))

    out = np.zeros((G, HID), np.float32)
    for c in range(C):
        out += np.asarray(rb.results[c]["pooled"])
    return out



# revision 2
# speedup vs baseline: 1.0297x; 1.0297x over previous
"""RGCN (2-layer, mean aggr) + global mean pool on 8 TRN2 NeuronCores.

Sharding: nodes split contiguously across 8 cores; within each core nodes are
permuted into 10 balanced groups of <=128 so that every (group, relation)
window holds <=t_w*128 incoming edges.  Segment sums run on the tensor engine
as fp8 DoubleRow matmuls g_t.T @ S_t per window, producing meanT [dim, node]
directly (S holds compensated fp8 1/count weights, so PSUM is the mean).
Phase A computes layer-1 means and the dense layer-1 hidden h (scaled by S1
to keep fp8 planes in range) for owned nodes; the host all-gathers h into an
fp8 table.  Phase B gathers layer-2 messages with batched dma_gather (<=1024
rows x 512B per instruction), aggregates, applies the relation einsum + root
+ bias with hi+lo fp8 DoubleRow weight planes (scale S2), relus, and pools
per-graph partials (1/graph-count and 1/(S1*S2) folded into the pool
selector); the host sums the 8 partials.
"""

import numpy as np
import ml_dtypes

import concourse.bacc as bacc
import concourse.bass as bass
import concourse.mybir as mybir
import concourse.tile as tile
from concourse.bass_utils import run_bass_kernel_spmd

N = 10000
E = 160000
R = 4
IN = 15
HID = 512
G = 64
C = 8
NPC = N // C            # 1250 nodes per core
RANGES = 10             # node groups of <=128 per core
NPAD = RANGES * 128     # 1280
NTAB = NPAD * C         # h table rows (virtual positions, 10240)
S1 = 8.0                # layer-1 output scale (keeps fp8 lo-planes normal)

F32 = mybir.dt.float32
BF16 = mybir.dt.bfloat16
FP8 = mybir.dt.float8e4
I16 = mybir.dt.int16
DRow = mybir.MatmulPerfMode.DoubleRow
Relu = mybir.ActivationFunctionType.Relu
Copy = mybir.ActivationFunctionType.Copy

NP_FP8 = ml_dtypes.float8_e4m3
NP_BF16 = ml_dtypes.bfloat16

_CACHE = {}
_PREP = {}


def _fp8(a):
    return np.asarray(a, np.float32).astype(NP_FP8)


def _hi_lo(w, scale):
    hi = _fp8(w * scale)
    lo = _fp8(w * scale - hi.astype(np.float32))
    return hi, lo


# ---------------------------------------------------------------- host prep
def _pack_core(deg, cap, kind, tie, seed):
    """Assign one core's nodes to RANGES groups: size<=128, per-rel sum<=cap.
    deg: [NPC, R] int. Returns assign [NPC] or None if infeasible."""
    tot = deg.sum(1)
    if seed is None:
        order = np.argsort(-tot, kind=kind)
    else:
        rng = np.random.default_rng(seed)
        jitter = rng.random(len(tot))
        order = np.lexsort((jitter, -tot))
    gsum = np.zeros((RANGES, R), np.int64)
    gcnt = np.zeros(RANGES, np.int64)
    assign = np.empty(len(deg), np.int64)
    for i in order:
        best = None
        cand = -1
        for g in range(RANGES):
            if gcnt[g] >= 128:
                continue
            ns = gsum[g] + deg[i]
            if (ns > cap).any():
                continue
            score = (ns / cap).max() + gcnt[g] * tie
            if best is None or score < best:
                best = score
                cand = g
        if cand < 0:
            return None
        assign[i] = cand
        gsum[cand] += deg[i]
        gcnt[cand] += 1
    return assign


_PACK_VARIANTS = [("quicksort", 1e-4, None), ("stable", 0.0, None),
                  ("stable", 1e-4, None), ("quicksort", 0.0, None),
                  ("stable", 1e-4, 0), ("stable", 1e-4, 1),
                  ("stable", 1e-4, 2), ("stable", 1e-4, 3)]


def _prep_structure(edge_index, edge_type, batch):
    src = np.asarray(edge_index[0], dtype=np.int64)
    tgt = np.asarray(edge_index[1], dtype=np.int64)
    rel = np.asarray(edge_type, dtype=np.int64)
    batch = np.asarray(batch, dtype=np.int64)

    key = hash((src.tobytes(), tgt.tobytes(), rel.tobytes(), batch.tobytes()))
    if key in _PREP:
        return _PREP[key]

    deg = np.zeros((N, R), np.int64)
    np.add.at(deg, (tgt, rel), 1)

    # node -> (range group, column) per core, balanced so windows fit t_w tiles
    for t_w in (4, 5, 6):
        cap = t_w * 128
        rg_of = np.empty(N, np.int64)
        ok = True
        for c in range(C):
            a = None
            for kind, tie, seed in _PACK_VARIANTS:
                a = _pack_core(deg[c * NPC:(c + 1) * NPC], cap, kind, tie, seed)
                if a is not None:
                    break
            if a is None:
                ok = False
                break
            rg_of[c * NPC:(c + 1) * NPC] = a
        if ok:
            break
    assert ok, "window packing failed"
    TPR = R * t_w               # tiles per range
    TILES = RANGES * TPR        # tiles per core
    SLOTS = TILES * 128

    # column within group: stable order by node id
    col_of = np.empty(N, np.int64)
    for c in range(C):
        ids = np.arange(c * NPC, (c + 1) * NPC)
        o = np.lexsort((ids, rg_of[ids]))
        sid = ids[o]
        grp = rg_of[sid]
        start = np.searchsorted(grp, np.arange(RANGES))
        col_of[sid] = np.arange(NPC) - start[grp]
    assert (col_of < 128).all()
    vpos = rg_of * 128 + col_of          # virtual position within core [0,1280)

    # edge -> slot
    ecore = tgt // NPC
    win = (ecore * RANGES + rg_of[tgt]) * R + rel   # global window id
    order = np.lexsort((src, win))
    swin = win[order]
    ssrc = src[order]
    scol = col_of[tgt][order]
    wcount = np.bincount(win, minlength=C * RANGES * R)
    assert wcount.max() <= t_w * 128
    wstart = np.zeros(C * RANGES * R + 1, np.int64)
    np.cumsum(wcount, out=wstart[1:])
    pos = np.arange(E) - wstart[swin]
    # gather index within core: (rg*TPR + rel*t_w + pos//128)*128 + pos%128
    lrg = (swin // R) % RANGES
    lrel = swin % R
    gidx = (lrg * TPR + lrel * t_w + pos // 128) * 128 + pos % 128
    score = swin // (RANGES * R)

    # compensated fp8 segment-mean weights: every edge of (tgt,rel) gets
    # fp8(1/cnt); the group's first edge gets fp8(1 - (cnt-1)*fp8(1/cnt)) so
    # the weights sum to ~1 exactly.
    cnt = np.bincount(tgt * R + rel, minlength=N * R)
    inv_q = np.zeros(N * R, np.float32)
    nz = cnt > 0
    inv_q[nz] = _fp8(1.0 / cnt[nz]).astype(np.float32)
    seg_o = (tgt[order] * R + rel[order]).astype(np.int64)
    sval = inv_q[seg_o]
    _, first_idx = np.unique(seg_o, return_index=True)
    segf = seg_o[first_idx]
    sval[first_idx] = _fp8(
        1.0 - (cnt[segf] - 1) * inv_q[segf]).astype(np.float32)

    # gather source row: virtual table row = core(src)*NPAD + vpos(src)
    svrow = (ssrc // NPC) * NPAD + vpos[ssrc]

    idx16 = np.zeros((C, 16, SLOTS // 16), np.int16)
    S = np.zeros((C, 128, TILES, 128), NP_FP8)
    for c in range(C):
        m = score == c
        gi = gidx[m]
        idxc = np.zeros(SLOTS, np.int16)
        idxc[gi] = svrow[m].astype(np.int16)
        idx16[c] = idxc.reshape(SLOTS // 16, 16).T
        S[c, gi % 128, gi // 128, scol[m]] = sval[m].astype(NP_FP8)
    idx128 = np.ascontiguousarray(np.tile(idx16, (1, 8, 1)))   # [C,128,SL/16]

    # poolS with 1/graph-size and fp8-plane unscale folded in
    gcnt_g = np.bincount(batch, minlength=G)
    ginv = np.zeros(G, np.float64)
    ginv[gcnt_g > 0] = 1.0 / gcnt_g[gcnt_g > 0]
    poolS = np.zeros((C, 128, RANGES, G), np.float64)
    nid = np.arange(N)
    poolS[nid // NPC, col_of, rg_of, batch] = ginv[batch]

    st = dict(t_w=t_w, TPR=TPR, TILES=TILES, SLOTS=SLOTS,
              rg_of=rg_of, col_of=col_of, vpos=vpos,
              idx128=idx128, S=S, gidx=gidx, score=score, ssrc=ssrc,
              poolS=poolS)
    _PREP.clear()
    _PREP[key] = st
    return st


# ---------------------------------------------------------------- phase A
def _build_phase_a(t_w):
    TPR = R * t_w
    TILES = RANGES * TPR
    nc = bacc.Bacc("TRN2", target_bir_lowering=True)
    xg_d = nc.dram_tensor("xg", [128, TILES * 16], FP8, kind="ExternalInput")
    s_d = nc.dram_tensor("stab", [128, TILES * 128], FP8, kind="ExternalInput")
    xoT_d = nc.dram_tensor("xoT", [16, NPAD], BF16, kind="ExternalInput")
    w1x_d = nc.dram_tensor("w1x", [16, HID], BF16, kind="ExternalInput")
    w1r_d = nc.dram_tensor("w1r", [16, 2 * R * HID], FP8, kind="ExternalInput")
    hT_d = nc.dram_tensor("hT", [128, 4 * NPAD], FP8, kind="ExternalOutput")

    with tile.TileContext(nc) as tc:
        with (
            tc.tile_pool(name="singles", bufs=1) as singles,
            tc.tile_pool(name="m1", bufs=5) as m1pool,
            tc.tile_pool(name="psA", bufs=4, space="PSUM") as psa_pool,
            tc.tile_pool(name="psH", bufs=3, space="PSUM") as psh_pool,
        ):
            xg = singles.tile([128, TILES, 16], FP8)
            s_sb = singles.tile([128, TILES, 128], FP8)
            nc.sync.dma_start(
                out=xg[:, :TPR, :],
                in_=xg_d[:, :TPR * 16].rearrange("p (t c) -> p t c", c=16))
            xoT = singles.tile([16, NPAD], BF16)
            w1x = singles.tile([16, HID], BF16)
            w1r = singles.tile([16, 2 * R, HID], FP8)
            hT = singles.tile([128, NPAD, 4], FP8)
            # per-range S loads so range-0 aggregation starts early; the
            # small dense-h operands go right after range 0's slab
            for rg in range(RANGES):
                nc.sync.dma_start(
                    out=s_sb[:, rg * TPR:(rg + 1) * TPR, :],
                    in_=s_d[:, rg * TPR * 128:(rg + 1) * TPR * 128]
                    .rearrange("p (t c) -> p t c", c=128))
                if rg == 0:
                    nc.sync.dma_start(out=xoT[:], in_=xoT_d[:])
                    nc.sync.dma_start(out=w1x[:], in_=w1x_d[:])
                    nc.sync.dma_start(
                        out=w1r[:],
                        in_=w1r_d[:].rearrange("p (r o) -> p r o", o=HID))
                elif rg == 1:
                    nc.sync.dma_start(
                        out=xg[:, TPR:, :],
                        in_=xg_d[:, TPR * 16:]
                        .rearrange("p (t c) -> p t c", c=16))

            def agg(rg):
                psA = psa_pool.tile([16, R, 128], F32, tag="psA")
                k0 = rg * TPR
                nmm = R * (t_w // 2 + t_w % 2)
                i_mm = 0
                for r in range(R):
                    kw = k0 + r * t_w
                    for p in range(t_w // 2):
                        nc.tensor.matmul(
                            psA[:, r, :],
                            lhsT=xg[:, kw + 2 * p:kw + 2 * p + 2, :],
                            rhs=s_sb[:, kw + 2 * p:kw + 2 * p + 2, :],
                            start=(i_mm == 0), stop=(i_mm == nmm - 1),
                            perf_mode=DRow)
                        i_mm += 1
                    if t_w % 2:
                        nc.tensor.matmul(
                            psA[:, r, :],
                            lhsT=xg[:, kw + t_w - 1, :],
                            rhs=s_sb[:, kw + t_w - 1, :],
                            start=(i_mm == 0), stop=(i_mm == nmm - 1))
                        i_mm += 1
                m1 = m1pool.tile([16, R, 128], FP8, tag="m1")
                nc.vector.tensor_copy(out=m1[:], in_=psA[:])
                return m1

            def dense(rg, m1):
                psH = psh_pool.tile([128, 4, 128], F32, tag="psH")
                for cch in range(4):
                    nc.tensor.matmul(
                        psH[:, cch, :],
                        lhsT=w1x[:, cch * 128:(cch + 1) * 128],
                        rhs=xoT[:, rg * 128:(rg + 1) * 128],
                        start=(cch == 0), stop=False)
                for cch in range(4):
                    for pl in range(2):          # hi, lo weight planes
                        for i in range(2):
                            nc.tensor.matmul(
                                psH[:, cch, :],
                                lhsT=w1r[:, 4 * pl + 2 * i:4 * pl + 2 * i + 2,
                                         cch * 128:(cch + 1) * 128],
                                rhs=m1[:, 2 * i:2 * i + 2, :],
                                start=False,
                                stop=(cch == 3 and pl == 1 and i == 1),
                                perf_mode=DRow)
                nc.scalar.activation(
                    hT[:, rg * 128:(rg + 1) * 128, :],
                    psH[:].rearrange("p c n -> p n c"), Relu)
                # stream this range's h slice out (512B-contiguous rows)
                nc.sync.dma_start(
                    out=hT_d[:, rg * 512:(rg + 1) * 512],
                    in_=hT[:, rg * 128:(rg + 1) * 128, :]
                    .rearrange("p n c -> p (n c)"))

            # software pipeline: dense(rg-1) queued on PE after agg(rg), so
            # the PE never stalls on the DVE mean-copy
            m1s = {}
            for rg in range(RANGES):
                with tc.tile_wait_until(rg * 1.0):
                    m1s[rg] = agg(rg)
                if rg >= 1:
                    with tc.tile_wait_until(rg * 1.0 + 0.5):
                        dense(rg - 1, m1s.pop(rg - 1))
            with tc.tile_wait_until(RANGES * 1.0 + 0.5):
                dense(RANGES - 1, m1s.pop(RANGES - 1))
    nc.compile()
    return nc


# ---------------------------------------------------------------- phase B
def _build_phase_b(t_w, has_b2):
    TPR = R * t_w
    TILES = RANGES * TPR
    SLOTS = TILES * 128
    GT = t_w * max(1, 8 // t_w)     # tiles per gather (<=1024 idxs)
    NGR = TPR // GT                 # gathers per range
    WPG = GT // t_w                 # windows per gather
    nc = bacc.Bacc("TRN2", target_bir_lowering=True)
    htab_d = nc.dram_tensor("htab", [NTAB, HID], FP8, kind="ExternalInput")
    idx_d = nc.dram_tensor("idx", [128, SLOTS // 16], I16, kind="ExternalInput")
    s_d = nc.dram_tensor("stab", [128, TILES * 128], FP8, kind="ExternalInput")
    w2f_d = nc.dram_tensor("w2f", [128, 32 * HID], FP8, kind="ExternalInput")
    w2r_d = nc.dram_tensor("w2r", [128, 8 * HID], FP8, kind="ExternalInput")
    hTo_d = nc.dram_tensor("hTo", [128, 4 * NPAD], FP8, kind="ExternalInput")
    pS_d = nc.dram_tensor("poolS", [128, RANGES * G], BF16, kind="ExternalInput")
    b2_d = (nc.dram_tensor("b2row", [1, HID], BF16, kind="ExternalInput")
            if has_b2 else None)
    out_d = nc.dram_tensor("pooled", [G, HID], F32, kind="ExternalOutput")

    with tile.TileContext(nc) as tc:
        with (
            tc.tile_pool(name="singles", bufs=1) as singles,
            tc.tile_pool(name="gb", bufs=8) as gpool,
            tc.tile_pool(name="mt", bufs=12) as mtpool,
            tc.tile_pool(name="o2", bufs=4) as o2pool,
            tc.tile_pool(name="pm", bufs=4, space="PSUM") as pmpool,
            tc.tile_pool(name="po", bufs=3, space="PSUM") as popool,
            tc.tile_pool(name="pp", bufs=1, space="PSUM") as pppool,
        ):
            s_sb = singles.tile([128, TILES, 128], FP8)
            idx_sb = singles.tile([128, SLOTS // 16], I16)
            w2f = singles.tile([128, 32, HID], FP8)
            w2r = singles.tile([128, 8, HID], FP8)
            hTo = singles.tile([128, 4, NPAD], FP8)
            pS = singles.tile([128, RANGES, G], BF16)
            if has_b2:
                b2 = singles.tile([1, HID], BF16)
                ones = singles.tile([1, 128], BF16)
                nc.vector.memset(ones[:], 1.0)

            def load_s(rg):
                nc.sync.dma_start(
                    out=s_sb[:, rg * TPR:(rg + 1) * TPR, :],
                    in_=s_d[:, rg * TPR * 128:(rg + 1) * TPR * 128]
                    .rearrange("p (t c) -> p t c", c=128))

            # idx slabs first (gate the gathers), then S for ranges 0-1;
            # the einsum operands trickle in behind the first gathers
            nc.sync.dma_start(out=idx_sb[:, :TPR * 8],
                              in_=idx_d[:, :TPR * 8])
            load_s(0)
            nc.sync.dma_start(out=idx_sb[:, TPR * 8:],
                              in_=idx_d[:, TPR * 8:])
            load_s(1)

            def load_rest(rg):
                if rg + 2 < RANGES:
                    load_s(rg + 2)
                if rg == 0:
                    nc.sync.dma_start(
                        out=w2f[:, 0:16, :],
                        in_=w2f_d[:, :16 * HID]
                        .rearrange("p (k o) -> p k o", o=HID))
                    nc.sync.dma_start(
                        out=w2r[:],
                        in_=w2r_d[:].rearrange("p (k o) -> p k o", o=HID))
                    if has_b2:
                        nc.sync.dma_start(out=b2[:], in_=b2_d[:])
                elif rg == 1:
                    nc.sync.dma_start(
                        out=w2f[:, 16:32, :],
                        in_=w2f_d[:, 16 * HID:]
                        .rearrange("p (k o) -> p k o", o=HID))
                    nc.sync.dma_start(
                        out=hTo[:],
                        in_=hTo_d[:].rearrange("p (c n) -> p c n", n=NPAD))
                    nc.sync.dma_start(
                        out=pS[:],
                        in_=pS_d[:].rearrange("p (r g) -> p r g", g=G))

            pool_ps = pppool.tile([G, HID], F32)

            def gather_agg(rg):
                mts = []
                for gg in range(NGR):
                    gt = gpool.tile([128, GT, HID], FP8, tag="g")
                    col0 = (rg * TPR + gg * GT) * 8
                    nc.gpsimd.dma_gather(
                        out_ap=gt[:],
                        in_ap=htab_d[:, :],
                        idxs_ap=idx_sb[:, col0:col0 + GT * 8],
                        num_idxs=GT * 128,
                        num_idxs_reg=GT * 128,
                        elem_size=HID)
                    for wr in range(WPG):
                        r = gg * WPG + wr
                        pm = pmpool.tile([128, 4, 128], F32, tag="pm")
                        kw = rg * TPR + r * t_w
                        kl = wr * t_w
                        nmm = 4 * (t_w // 2 + t_w % 2)
                        i_mm = 0
                        for cch in range(4):
                            for p in range(t_w // 2):
                                nc.tensor.matmul(
                                    pm[:, cch, :],
                                    lhsT=gt[:, kl + 2 * p:kl + 2 * p + 2,
                                            cch * 128:(cch + 1) * 128],
                                    rhs=s_sb[:, kw + 2 * p:kw + 2 * p + 2, :],
                                    start=(i_mm == 0), stop=(i_mm == nmm - 1),
                                    perf_mode=DRow)
                                i_mm += 1
                            if t_w % 2:
                                nc.tensor.matmul(
                                    pm[:, cch, :],
                                    lhsT=gt[:, kl + t_w - 1,
                                            cch * 128:(cch + 1) * 128],
                                    rhs=s_sb[:, kw + t_w - 1, :],
                                    start=(i_mm == 0), stop=(i_mm == nmm - 1))
                                i_mm += 1
                        mt = mtpool.tile([128, 4, 128], FP8, tag=f"mt{r}")
                        if r % 2 == 0:
                            nc.vector.tensor_copy(out=mt[:], in_=pm[:])
                        else:
                            nc.scalar.activation(mt[:], pm[:], Copy)
                        mts.append(mt)
                return mts

            def einsum(rg, mts):
                po = popool.tile([128, HID], F32, tag="po")
                first = True
                for pl in range(2):              # hi, lo weight planes
                    for r in range(R):
                        for i in range(2):
                            nc.tensor.matmul(
                                po[:], lhsT=mts[r][:, 2 * i:2 * i + 2, :],
                                rhs=w2f[:, 16 * pl + 4 * r + 2 * i:
                                        16 * pl + 4 * r + 2 * i + 2, :],
                                start=first, stop=False, perf_mode=DRow)
                            first = False
                for pl in range(2):
                    for i in range(2):
                        nc.tensor.matmul(
                            po[:],
                            lhsT=hTo[:, 2 * i:2 * i + 2,
                                     rg * 128:(rg + 1) * 128],
                            rhs=w2r[:, 4 * pl + 2 * i:4 * pl + 2 * i + 2, :],
                            start=False,
                            stop=(pl == 1 and i == 1 and not has_b2),
                            perf_mode=DRow)
                if has_b2:
                    nc.tensor.matmul(po[:], lhsT=ones[:], rhs=b2[:],
                                     start=False, stop=True)
                o2 = o2pool.tile([128, HID], BF16, tag="o2")
                nc.scalar.activation(o2[:], po[:], Relu)
                return o2

            def pool(rg, o2):
                nc.tensor.matmul(pool_ps[:], lhsT=pS[:, rg, :], rhs=o2[:],
                                 start=(rg == 0), stop=(rg == RANGES - 1))

            # software pipeline: einsum at lag 1, pool at lag 2, so the PE
            # never stalls on mean-copies (DVE/Act) or the relu (Act)
            mts_all, o2_all = {}, {}
            for rg in range(RANGES):
                with tc.tile_wait_until(rg * 1.0):
                    mts_all[rg] = gather_agg(rg)
                    load_rest(rg)
                if rg >= 1:
                    with tc.tile_wait_until(rg * 1.0 + 0.5):
                        o2_all[rg - 1] = einsum(rg - 1, mts_all.pop(rg - 1))
                if rg >= 2:
                    with tc.tile_wait_until(rg * 1.0 + 0.7):
                        pool(rg - 2, o2_all.pop(rg - 2))
            with tc.tile_wait_until(RANGES * 1.0 + 0.5):
                o2_all[RANGES - 1] = einsum(RANGES - 1,
                                            mts_all.pop(RANGES - 1))
            with tc.tile_wait_until(RANGES * 1.0 + 0.7):
                pool(RANGES - 2, o2_all.pop(RANGES - 2))
                pool(RANGES - 1, o2_all.pop(RANGES - 1))
            pooled = o2pool.tile([G, HID], F32, tag="pooled")
            nc.vector.tensor_copy(out=pooled[:], in_=pool_ps[:])
            nc.sync.dma_start(out=out_d[:], in_=pooled[:])
    nc.compile()
    return nc


# ---------------------------------------------------------------- driver
def kernel(x, edge_index, edge_type, batch, W1_rel, W1_root, b1,
           W2_rel, W2_root, b2, _collect_times=None):
    import time as _time
    x = np.asarray(x, np.float32)
    W1_rel = np.asarray(W1_rel, np.float32)
    W1_root = np.asarray(W1_root, np.float32)
    b1 = np.asarray(b1, np.float32)
    W2_rel = np.asarray(W2_rel, np.float32)
    W2_root = np.asarray(W2_root, np.float32)
    b2 = np.asarray(b2, np.float32)

    st = _prep_structure(edge_index, edge_type, batch)
    t_w = st["t_w"]
    TILES = st["TILES"]

    has_b2 = bool(np.any(b2 != 0))
    if ("A", t_w) not in _CACHE:
        _CACHE[("A", t_w)] = _build_phase_a(t_w)
    if ("B", t_w, has_b2) not in _CACHE:
        _CACHE[("B", t_w, has_b2)] = _build_phase_b(t_w, has_b2)
    nca, ncb = _CACHE[("A", t_w)], _CACHE[("B", t_w, has_b2)]

    # ---- phase A inputs
    xpad = np.zeros((N, 16), np.float32)
    xpad[:, :IN] = x
    ssrc, score, gidx = st["ssrc"], st["score"], st["gidx"]
    vpos = st["vpos"]

    w1x = np.zeros((16, HID), np.float32)
    w1x[0:IN] = W1_root
    w1x[IN] = b1
    w1rs = np.zeros((16, R, HID), np.float32)
    w1rs[0:IN] = W1_rel.transpose(1, 0, 2)
    w1r_hi, w1r_lo = _hi_lo(w1rs, S1)
    w1r_in = np.ascontiguousarray(
        np.concatenate([w1r_hi, w1r_lo], axis=1).reshape(16, 2 * R * HID))

    in_maps_a = []
    for c in range(C):
        m = score == c
        gi = gidx[m]
        xg = np.zeros((128, TILES, 16), np.float32)
        xg[gi % 128, gi // 128] = xpad[ssrc[m]]
        xoT = np.zeros((16, NPAD), np.float32)
        own = np.arange(c * NPC, (c + 1) * NPC)
        xoT[:IN, vpos[own]] = x[own].T
        xoT[IN] = 1.0
        in_maps_a.append({
            "xg": np.ascontiguousarray(
                xg.reshape(128, TILES * 16)).astype(NP_FP8),
            "stab": np.ascontiguousarray(
                st["S"][c].reshape(128, TILES * 128)),
            "xoT": xoT.astype(NP_BF16),
            "w1x": (w1x * S1).astype(NP_BF16),
            "w1r": w1r_in,
        })
    _t0 = _time.time()
    ra = run_bass_kernel_spmd(nca, in_maps_a, core_ids=list(range(C)))
    if _collect_times is not None:
        _collect_times.append(int((_time.time() - _t0) * 1e9))

    # ---- host exchange: assemble h table (virtual rows) + fp8 copies
    htab = np.zeros((NTAB, HID), NP_FP8)
    hTo_in = []
    for c in range(C):
        # fp8 [128, NPAD*4] with element (kp, n*4+c) = h'[node n, c*128+kp]
        hT = np.asarray(ra.results[c]["hT"]).reshape(128, NPAD, 4)
        htab[c * NPAD:(c + 1) * NPAD] = np.ascontiguousarray(
            hT.transpose(1, 2, 0)).reshape(NPAD, HID)
        hTo_in.append(np.ascontiguousarray(
            hT.transpose(0, 2, 1)).reshape(128, 4 * NPAD))

    # W2 planes at scale S2 (power of two fitting fp8 range)
    wmax = max(np.abs(W2_rel).max(), np.abs(W2_root).max(), 1e-30)
    S2 = float(2.0 ** np.floor(np.log2(224.0 / wmax)))
    w2fs = np.zeros((128, 16, HID), np.float32)
    for r in range(R):
        for cch in range(4):
            w2fs[:, 4 * r + cch] = W2_rel[r, cch * 128:(cch + 1) * 128]
    w2rs = np.zeros((128, 4, HID), np.float32)
    for cch in range(4):
        w2rs[:, cch] = W2_root[cch * 128:(cch + 1) * 128]
    w2f_hi, w2f_lo = _hi_lo(w2fs, S2)
    w2r_hi, w2r_lo = _hi_lo(w2rs, S2)
    w2f_in = np.ascontiguousarray(
        np.concatenate([w2f_hi, w2f_lo], axis=1).reshape(128, 32 * HID))
    w2r_in = np.ascontiguousarray(
        np.concatenate([w2r_hi, w2r_lo], axis=1).reshape(128, 8 * HID))
    unscale = 1.0 / (S1 * S2)
    pS_in = np.ascontiguousarray(
        (st["poolS"] * unscale).astype(NP_BF16).reshape(C, 128, RANGES * G))
    b2_in = (b2 * S1 * S2).reshape(1, HID).astype(NP_BF16)

    in_maps_b = []
    for c in range(C):
        in_maps_b.append({
            "htab": htab,
            "idx": st["idx128"][c],
            "stab": np.ascontiguousarray(
                st["S"][c].reshape(128, TILES * 128)),
            "w2f": w2f_in,
            "w2r": w2r_in,
            "hTo": hTo_in[c],
            "poolS": pS_in[c],
        })
        if has_b2:
            in_maps_b[-1]["b2row"] = b2_in
    _t0 = _time.time()
    rb = run_bass_kernel_spmd(ncb, in_maps_b, core_ids=list(range(C)))
    if _collect_times is not None:
        _collect_times.append(int((_time.time() - _t0) * 1e9))

    out = np.zeros((G, HID), np.float32)
    for c in range(C):
        out += np.asarray(rb.results[c]["pooled"])
    return out
